# revision 4
# baseline (speedup 1.0000x reference)
"""Trainium2 Bass kernel for nn_CoverageLoss.

Math: the reference loss per fragment point is

    min over boxes b of ( min-dist^2 to 100 boundary samples of b ) * outside(b)

The 100 boundary samples are 25 uniformly-spaced points (t = k/24) on each of
the 4 box edges, so the min over samples of one edge has a closed form via
clamped rounding: for the two vertical edges the x-term is (|fx-xc| - w/2)^2
and the y-term is cy^2 with cy = dyl - clamp(round(24*dyl/h), 0, 24) * h/24.
min_b (dist_b * outside_b) == 0 if the point is inside any box, else the plain
min of distances - so the mask becomes "min with BIG*(slack_x+slack_y)" folded
into the overall min (slack_a = relu(|f_a - ctr_a| - half_a), zero iff inside
that slab).

Sharding: data-parallel over images; core k handles images [4k, 4k+4) and
their 32 boxes.  Per core the 32768 (point, box) pairs are laid out as
[128 partitions = (box b:8 outer, q=(image n:4, chunk c:4)), 256 points]
fp32 tiles; fragment coords arrive pre-replicated to the 8 b-row groups
(host-side layout) so one unit-stride DMA per coordinate loads them.

Schedule notes (all engine assignments deliberate):
  - boxp is DMA'd FIRST on the scalar queue (2KB, lands ~0.8us before the
    128KB fragment streams) so the DVE box-constant prep runs during the
    fragment DMA; fx rides the sync queue, fy second on scalar.
  - ACT does exactly 8 big ops (s0/au/e/qc per axis); the mask ops
    (zz per axis + zsum) run on the otherwise-idle gpsimd engine.
  - dz is emitted in bf16 so the two PE transposes and the box-min reduce
    run at 2x; the final per-partition sum [128,1] is DMA'd out and the
    host adds 8*128 partials (the unshard of the mean).
  - The NEFF teardown clears every semaphore the compiler *could* have
    used one EVENT_SEMAPHORE at a time (253 clears across 5 engines,
    ~4us inside the measured window); --max-sem-num caps the space so
    the teardown shrinks to the handful of semaphores actually used.
"""

import os
import numpy as np
from contextlib import ExitStack

import concourse.bass as bass
import concourse.bacc as bacc
import concourse.tile as tile
from concourse import masks, mybir
import concourse.bass_utils as _BU
from concourse.bass_utils import run_bass_kernel_spmd

# problem shape (hardcoded per the harness contract)
N_CORES = 8
N_IMG = 32            # total images
NI = N_IMG // N_CORES  # images per core = 4
BPI = 8               # boxes per image
F, FP = 16, 64        # fragments per image, points per fragment
PTS = F * FP          # 1024 points per image
CH = 4                # chunks per image
CW = PTS // CH        # 256 points per chunk
ROWS = NI * CH        # 16 (n, c) rows

DELTA = np.float32(1.0 / 24.0)
MAGIC = float(np.float32(2.0 ** 23))
BIG = float(np.float32(1.0e30))

FP32 = mybir.dt.float32
BF16 = mybir.dt.bfloat16
OP = mybir.AluOpType
AF = mybir.ActivationFunctionType


def _env(name, default):
    v = os.environ.get(name)
    return default if v is None else v


# Cap the semaphore space the backend may allocate from.  The NEFF epilogue
# resets the whole allocatable range one semaphore at a time, which is the
# bulk of the fixed tail; the kernel itself needs < 24 semaphores.
_MSN = _env("COV_MSN", "32")
_BF16_TAIL = _env("COV_BF16_TAIL", "1") == "1"

_walrus_patched = False


def _patch_walrus_args():
    global _walrus_patched
    if _walrus_patched or not _MSN or _MSN == "0":
        return
    _walrus_patched = True
    orig = _BU.get_walrus_args

    def patched(*a, **k):
        return list(orig(*a, **k)) + [f"--max-sem-num={_MSN}"]

    _BU.get_walrus_args = patched


def build_nc():
    nc = bacc.Bacc("TRN2", target_bir_lowering=False, debug=False)

    frag2 = nc.dram_tensor("frag2", [2, 128, CW], FP32, kind="ExternalInput").ap()
    boxp = nc.dram_tensor("boxp", [128, 4], FP32, kind="ExternalInput").ap()
    out = nc.dram_tensor("out", [128, 1], FP32, kind="ExternalOutput").ap()

    tdt = BF16 if _BF16_TAIL else FP32

    with tile.TileContext(nc) as tc:
        with ExitStack() as ctx:
            pool = ctx.enter_context(tc.tile_pool(name="main", bufs=1))

            def t128(tag, w=CW, dt=FP32):
                return pool.tile([128, w], dt, tag=tag, name=tag)

            # ---- input DMAs: boxp first (tiny, gates the prep chain) ----
            bx = pool.tile([128, 4], FP32, tag="bx", name="bx")
            fx = t128("fx")
            fy = t128("fy")
            nc.scalar.dma_start(bx[:], boxp[:])
            nc.sync.dma_start(fx[:], frag2[0])
            nc.scalar.dma_start(fy[:], frag2[1])

            # ---- transpose identity (gpsimd, off critical path) ----
            idn = pool.tile([128, 128], tdt, tag="idn", name="idn")
            masks.make_identity(nc, idn[:])

            # ---- box scalar prep ([128, 2] columns = x, y axis) ----
            # ordered so the s0 dependencies (winv, nlw) finish first
            ctr = bx[:, 0:2]
            sz = bx[:, 2:4]
            rec = pool.tile([128, 2], FP32, tag="rec", name="rec")      # 1/w
            nc.vector.reciprocal(rec[:], sz)
            lo = pool.tile([128, 2], FP32, tag="lo", name="lo")         # ctr-0.5*sz
            nc.vector.scalar_tensor_tensor(lo[:], sz, -0.5, ctr,
                                           OP.mult, OP.add)
            winv = pool.tile([128, 2], FP32, tag="winv", name="winv")   # 24/w
            nc.vector.tensor_scalar(winv[:], rec[:], 24.0, None, OP.mult)
            nlw = pool.tile([128, 2], FP32, tag="nlw", name="nlw")      # -lo*winv
            nc.vector.scalar_tensor_tensor(nlw[:], lo[:], -1.0, winv[:],
                                           OP.mult, OP.mult)
            nctr = pool.tile([128, 2], FP32, tag="nctr", name="nctr")   # -ctr
            nc.vector.tensor_scalar(nctr[:], ctr, -1.0, None, OP.mult)
            nhalf = pool.tile([128, 2], FP32, tag="nhalf", name="nhalf")  # -sz/2
            nc.vector.tensor_scalar(nhalf[:], sz, -0.5, None, OP.mult)
            wd = pool.tile([128, 2], FP32, tag="wd", name="wd")         # sz/24
            nc.vector.tensor_scalar(wd[:], sz, float(DELTA), None, OP.mult)

            # ---- per-axis pipelines ----
            # ACT order: s0x, aux, s0y, auy, ex, qcx, ey, qcy  (8 ops)
            # DVE: q1/ox/tx per axis;  gpsimd: zz per axis + zsum
            def cs(col):
                return slice(col, col + 1)

            s0x = t128("s0x")
            nc.scalar.activation(s0x[:], fx[:], AF.Relu,
                                 bias=nlw[:, cs(0)], scale=winv[:, cs(0)])
            aux = t128("aux")
            nc.scalar.activation(aux[:], fx[:], AF.Abs, bias=nctr[:, cs(0)])
            s0y = t128("s0y")
            nc.scalar.activation(s0y[:], fy[:], AF.Relu,
                                 bias=nlw[:, cs(1)], scale=winv[:, cs(1)])
            auy = t128("auy")
            nc.scalar.activation(auy[:], fy[:], AF.Abs, bias=nctr[:, cs(1)])

            # DVE chains (x then y)
            q1x = t128("q1x")
            nc.vector.tensor_scalar(q1x[:], s0x[:], 24.0, MAGIC, OP.min, OP.add)
            oxx = t128("oxx")
            nc.vector.tensor_scalar(oxx[:], q1x[:], MAGIC, wd[:, cs(0)],
                                    OP.subtract, OP.mult)
            txx = t128("txx")
            nc.vector.tensor_tensor(txx[:], oxx[:], fx[:], OP.subtract)
            q1y = t128("q1y")
            nc.vector.tensor_scalar(q1y[:], s0y[:], 24.0, MAGIC, OP.min, OP.add)
            oyy = t128("oyy")
            nc.vector.tensor_scalar(oyy[:], q1y[:], MAGIC, wd[:, cs(1)],
                                    OP.subtract, OP.mult)
            tyy = t128("tyy")
            nc.vector.tensor_tensor(tyy[:], oyy[:], fy[:], OP.subtract)

            # gpsimd mask chain: slack per axis, then the sum
            zzx = t128("zzx")
            nc.gpsimd.tensor_scalar(zzx[:], aux[:], nhalf[:, cs(0)], 0.0,
                                    OP.add, OP.max)
            zzy = t128("zzy")
            nc.gpsimd.tensor_scalar(zzy[:], auy[:], nhalf[:, cs(1)], 0.0,
                                    OP.add, OP.max)
            zs = t128("zs")
            nc.gpsimd.tensor_tensor(zs[:], zzx[:], zzy[:], OP.add)

            # remaining ACT ops
            ex = t128("ex")
            nc.scalar.activation(ex[:], aux[:], AF.Square, bias=nhalf[:, cs(0)])
            qcx = t128("qcx")
            nc.scalar.activation(qcx[:], txx[:], AF.Square, bias=lo[:, cs(0)])
            ey = t128("ey")
            nc.scalar.activation(ey[:], auy[:], AF.Square, bias=nhalf[:, cs(1)])
            qcy = t128("qcy")
            nc.scalar.activation(qcy[:], tyy[:], AF.Square, bias=lo[:, cs(1)])

            # ---- combine (DVE tail) ----
            e2 = t128("e2")
            nc.vector.tensor_tensor(e2[:], ey[:], qcx[:], OP.add)
            m2 = t128("m2")   # min(BIG*zs, e2)
            nc.vector.scalar_tensor_tensor(m2[:], zs[:], BIG, e2[:],
                                           OP.mult, OP.min)
            e1 = t128("e1")
            nc.vector.tensor_tensor(e1[:], ex[:], qcy[:], OP.add)
            dzA = pool.tile([128, 128], tdt, tag="dzA", name="dzA")
            dzB = pool.tile([128, 128], tdt, tag="dzB", name="dzB")
            nc.vector.tensor_tensor(dzA[:], e1[:, 0:128], m2[:, 0:128], OP.min)
            nc.vector.tensor_tensor(dzB[:], e1[:, 128:256], m2[:, 128:256],
                                    OP.min)

            # ---- min over the 8 box rows ----
            # PE-transpose dz (points onto partitions); the box-min becomes a
            # strided free-dim reduce over one combined PSUM view, then one
            # more reduce sums the 32 (half, q) columns per point partition.
            with tc.tile_pool(name="psum", bufs=1, space="PSUM") as psum_pool:
                pAB = psum_pool.tile([128, 256], tdt, tag="pAB", name="pAB")
                nc.tensor.matmul(pAB[:, 0:128], dzA[:], idn[:],
                                 is_transpose=True)
                nc.tensor.matmul(pAB[:, 128:256], dzB[:], idn[:],
                                 is_transpose=True)
                mAB = pool.tile([128, 32], FP32, tag="mAB", name="mAB")
                nc.vector.tensor_reduce(
                    mAB.rearrange("p (h q) -> p h q", h=2),
                    pAB.rearrange("p (h b q) -> p h q b", h=2, b=BPI),
                    axis=mybir.AxisListType.X, op=OP.min)
                persum = pool.tile([128, 1], FP32, tag="persum", name="persum")
                nc.vector.tensor_reduce(
                    persum[:], mAB[:], axis=mybir.AxisListType.X, op=OP.add)
                nc.sync.dma_start(out[:], persum[:])

    nc.compile()
    return nc


# partition row p = b*16 + q, q = n*4 + c
_P = np.arange(128)
_B_IDX = _P // (NI * CH)
_N_IDX = (_P % (NI * CH)) // CH


def shard_inputs(boxes, fragments):
    """Per-core input marshalling (layout only, no arithmetic)."""
    boxes = np.ascontiguousarray(boxes, dtype=np.float32).reshape(
        N_CORES, NI, BPI, 4)
    frag = np.ascontiguousarray(fragments, dtype=np.float32).reshape(
        N_CORES, NI, CH, CW, 2)
    in_maps = []
    for k in range(N_CORES):
        f2 = frag[k].transpose(3, 0, 1, 2).reshape(2, ROWS, CW)
        frag2 = np.ascontiguousarray(
            np.broadcast_to(f2[:, None], (2, BPI, ROWS, CW)).reshape(2, 128, CW))
        boxp = np.ascontiguousarray(boxes[k, _N_IDX, _B_IDX, :])
        in_maps.append({"frag2": frag2, "boxp": boxp})
    return in_maps


_NC = None


def _get_nc():
    global _NC
    if _NC is None:
        _patch_walrus_args()
        _NC = build_nc()
    return _NC


def run(boxes, fragments, trace=False, **spmd_kwargs):
    nc = _get_nc()
    in_maps = shard_inputs(boxes, fragments)
    res = run_bass_kernel_spmd(nc, in_maps, list(range(N_CORES)),
                               trace=trace, **spmd_kwargs)
    total = np.float32(sum(
        np.asarray(r["out"], dtype=np.float32).sum(dtype=np.float32)
        for r in res.results))
    loss = np.float32(total / np.float32(FP * N_IMG))
    return loss, res


def kernel(boxes, fragments, obj_to_img):
    loss, _ = run(boxes, fragments)
    return loss


# revision 5
# speedup vs baseline: 1.6608x; 1.6608x over previous
"""Trainium2 Bass kernel for nn_CoverageLoss.

Math: the reference loss per fragment point is

    min over boxes b of ( min-dist^2 to 100 boundary samples of b ) * outside(b)

The 100 boundary samples are 25 uniformly-spaced points (t = k/24) on each of
the 4 box edges, so the min over samples of one edge has a closed form via
clamped rounding: for the two vertical edges the x-term is (|fx-xc| - w/2)^2
and the y-term is cy^2 with cy = dyl - clamp(round(24*dyl/h), 0, 24) * h/24.
min_b (dist_b * outside_b) == 0 if the point is inside any box, else the plain
min of distances - so the mask becomes "min with BIG*(slack_x+slack_y)" folded
into the overall min (slack_a = relu(|f_a - ctr_a| - half_a), zero iff inside
that slab).

Sharding: data-parallel over images; core k handles images [4k, 4k+4) and
their 32 boxes.  Per core the 32768 (point, box) pairs are laid out as
[128 partitions = (box b:8 outer, q=(image n:4, chunk c:4)), 256 points]
fp32 tiles; fragment coords arrive pre-replicated to the 8 b-row groups
(host-side layout) so one unit-stride DMA per coordinate loads them.

Schedule notes (all engine assignments deliberate):
  - boxp is DMA'd FIRST on the sync queue as a single packet (2KB, lands
    ~0.8us before the 128KB fragment streams and does not wait for any
    straggler DMA engine) so the DVE box-constant prep overlaps the
    fragment DMAs; fx rides sync second, fy on scalar.
  - ACT does its 8 big ops (s0/au/e/qc per axis) plus three tiny prep
    scalings squeezed into its DMA-wait window; the mask chain
    (zz per axis + zs) and everything else elementwise runs on DVE.
    gpsimd only builds the transpose identity (its tensor ALU is ~8x
    slower than DVE and stalls DVE when used concurrently - measured).
  - dz is emitted in bf16 so the two PE transposes run at 2x.
  - The result leaves as a [1,32] row (point-partition sums from a
    ones-matmul): one contiguous DMA descriptor.  A [128,1] output
    generates 128 4-byte descriptors which take ~7us to retire.
"""

import os
import numpy as np
from contextlib import ExitStack

import concourse.bass as bass
import concourse.bacc as bacc
import concourse.tile as tile
from concourse import masks, mybir
import concourse.bass_utils as _BU
from concourse.bass_utils import run_bass_kernel_spmd

# problem shape (hardcoded per the harness contract)
N_CORES = 8
N_IMG = 32            # total images
NI = N_IMG // N_CORES  # images per core = 4
BPI = 8               # boxes per image
F, FP = 16, 64        # fragments per image, points per fragment
PTS = F * FP          # 1024 points per image
CH = 4                # chunks per image
CW = PTS // CH        # 256 points per chunk
ROWS = NI * CH        # 16 (n, c) rows

DELTA = np.float32(1.0 / 24.0)
MAGIC = float(np.float32(2.0 ** 23))
BIG = float(np.float32(1.0e30))

FP32 = mybir.dt.float32
BF16 = mybir.dt.bfloat16
OP = mybir.AluOpType
AF = mybir.ActivationFunctionType


def _env(name, default):
    v = os.environ.get(name)
    return default if v is None else v


_MSN = _env("COV_MSN", "0")           # --max-sem-num cap (0 = off)
_BF16_TAIL = _env("COV_BF16_TAIL", "1") == "1"
_BIRLOW = _env("COV_BIRLOW", "0") == "1"

_walrus_patched = False


def _patch_walrus_args():
    global _walrus_patched
    if _walrus_patched or not _MSN or _MSN == "0":
        return
    _walrus_patched = True
    orig = _BU.get_walrus_args

    def patched(*a, **k):
        return list(orig(*a, **k)) + [f"--max-sem-num={_MSN}"]

    _BU.get_walrus_args = patched


def build_nc():
    nc = bacc.Bacc("TRN2", target_bir_lowering=_BIRLOW, debug=False)

    frag2 = nc.dram_tensor("frag2", [2, 128, CW], FP32, kind="ExternalInput").ap()
    boxp = nc.dram_tensor("boxp", [128, 4], FP32, kind="ExternalInput").ap()
    out = nc.dram_tensor("out", [1, 32], FP32, kind="ExternalOutput").ap()

    tdt = BF16 if _BF16_TAIL else FP32

    with tile.TileContext(nc) as tc:
        with ExitStack() as ctx:
            pool = ctx.enter_context(tc.tile_pool(name="main", bufs=1))

            def t128(tag, w=CW, dt=FP32):
                return pool.tile([128, w], dt, tag=tag, name=tag)

            # ---- input DMAs: boxp first (tiny, gates the prep chain) ----
            bx = pool.tile([128, 4], FP32, tag="bx", name="bx")
            fx = t128("fx")
            fy = t128("fy")
            nc.sync.dma_start(bx[:], boxp[:], single_packet=True)
            nc.sync.dma_start(fx[:], frag2[0])
            nc.scalar.dma_start(fy[:], frag2[1])

            # ---- transpose identity + ones (gpsimd, off critical path) ----
            idn = pool.tile([128, 128], tdt, tag="idn", name="idn")
            masks.make_identity(nc, idn[:])
            ones = pool.tile([128, 1], FP32, tag="ones", name="ones")
            nc.gpsimd.memset(ones[:], 1.0)

            # ---- box scalar prep ([128, 2] columns = x, y axis) ----
            # DVE: the s0 dependencies (serial chain); ACT: the independent
            # scalings, squeezed in before fx lands.
            ctr = bx[:, 0:2]
            sz = bx[:, 2:4]
            rec = pool.tile([128, 2], FP32, tag="rec", name="rec")      # 1/w
            nc.vector.reciprocal(rec[:], sz)
            lo = pool.tile([128, 2], FP32, tag="lo", name="lo")         # ctr-sz/2
            nc.vector.scalar_tensor_tensor(lo[:], sz, -0.5, ctr,
                                           OP.mult, OP.add)
            winv = pool.tile([128, 2], FP32, tag="winv", name="winv")   # 24/w
            nc.vector.tensor_scalar(winv[:], rec[:], 24.0, None, OP.mult)
            nlw = pool.tile([128, 2], FP32, tag="nlw", name="nlw")      # -lo*winv
            nc.vector.scalar_tensor_tensor(nlw[:], lo[:], -1.0, winv[:],
                                           OP.mult, OP.mult)
            nctr = pool.tile([128, 2], FP32, tag="nctr", name="nctr")   # -ctr
            nc.scalar.activation(nctr[:], ctr, AF.Copy, scale=-1.0)
            nhalf = pool.tile([128, 2], FP32, tag="nhalf", name="nhalf")  # -sz/2
            nc.scalar.activation(nhalf[:], sz, AF.Copy, scale=-0.5)
            wd = pool.tile([128, 2], FP32, tag="wd", name="wd")         # sz/24
            nc.scalar.activation(wd[:], sz, AF.Copy, scale=float(DELTA))

            def cs(col):
                return slice(col, col + 1)

            # ---- ACT chain: 8 big ops ----
            s0x = t128("s0x")
            nc.scalar.activation(s0x[:], fx[:], AF.Relu,
                                 bias=nlw[:, cs(0)], scale=winv[:, cs(0)])
            aux = t128("aux")
            nc.scalar.activation(aux[:], fx[:], AF.Abs, bias=nctr[:, cs(0)])
            s0y = t128("s0y")
            nc.scalar.activation(s0y[:], fy[:], AF.Relu,
                                 bias=nlw[:, cs(1)], scale=winv[:, cs(1)])
            auy = t128("auy")
            nc.scalar.activation(auy[:], fy[:], AF.Abs, bias=nctr[:, cs(1)])
            ex = t128("ex")
            nc.scalar.activation(ex[:], aux[:], AF.Square, bias=nhalf[:, cs(0)])
            ey = t128("ey")
            nc.scalar.activation(ey[:], auy[:], AF.Square, bias=nhalf[:, cs(1)])

            # ---- DVE chains ----
            q1x = t128("q1x")
            nc.vector.tensor_scalar(q1x[:], s0x[:], 24.0, MAGIC, OP.min, OP.add)
            oxx = t128("oxx")
            nc.vector.tensor_scalar(oxx[:], q1x[:], MAGIC, wd[:, cs(0)],
                                    OP.subtract, OP.mult)
            txx = t128("txx")
            nc.vector.tensor_tensor(txx[:], oxx[:], fx[:], OP.subtract)
            q1y = t128("q1y")
            nc.vector.tensor_scalar(q1y[:], s0y[:], 24.0, MAGIC, OP.min, OP.add)
            oyy = t128("oyy")
            nc.vector.tensor_scalar(oyy[:], q1y[:], MAGIC, wd[:, cs(1)],
                                    OP.subtract, OP.mult)
            tyy = t128("tyy")
            nc.vector.tensor_tensor(tyy[:], oyy[:], fy[:], OP.subtract)
            # mask chain: slack per axis, then the sum
            zzx = t128("zzx")
            nc.vector.tensor_scalar(zzx[:], aux[:], nhalf[:, cs(0)], 0.0,
                                    OP.add, OP.max)
            zzy = t128("zzy")
            nc.vector.tensor_scalar(zzy[:], auy[:], nhalf[:, cs(1)], 0.0,
                                    OP.add, OP.max)
            zs = t128("zs")
            nc.vector.tensor_tensor(zs[:], zzx[:], zzy[:], OP.add)

            # last two ACT ops (need txx/tyy from DVE)
            qcx = t128("qcx")
            nc.scalar.activation(qcx[:], txx[:], AF.Square, bias=lo[:, cs(0)])
            qcy = t128("qcy")
            nc.scalar.activation(qcy[:], tyy[:], AF.Square, bias=lo[:, cs(1)])

            # ---- combine (DVE tail; m2 ready before qcy lands) ----
            e2 = t128("e2")
            nc.vector.tensor_tensor(e2[:], ey[:], qcx[:], OP.add)
            m2 = t128("m2")   # min(BIG*zs, e2)
            nc.vector.scalar_tensor_tensor(m2[:], zs[:], BIG, e2[:],
                                           OP.mult, OP.min)
            e1 = t128("e1")
            nc.vector.tensor_tensor(e1[:], ex[:], qcy[:], OP.add)
            dzA = pool.tile([128, 128], tdt, tag="dzA", name="dzA")
            dzB = pool.tile([128, 128], tdt, tag="dzB", name="dzB")
            nc.vector.tensor_tensor(dzA[:], e1[:, 0:128], m2[:, 0:128], OP.min)
            nc.vector.tensor_tensor(dzB[:], e1[:, 128:256], m2[:, 128:256],
                                    OP.min)

            # ---- min over the 8 box rows, then collapse ----
            # PE-transpose dz (points onto partitions); box-min = strided
            # free-dim reduce over the combined PSUM view; a ones-matmul
            # collapses the point partitions to one [1,32] row (the host
            # adds 8x32 partials - the unshard of the mean).
            with tc.tile_pool(name="psum", bufs=1, space="PSUM") as psum_pool:
                pAB = psum_pool.tile([128, 256], tdt, tag="pAB", name="pAB")
                nc.tensor.matmul(pAB[:, 0:128], dzA[:], idn[:],
                                 is_transpose=True)
                nc.tensor.matmul(pAB[:, 128:256], dzB[:], idn[:],
                                 is_transpose=True)
                mAB = pool.tile([128, 32], FP32, tag="mAB", name="mAB")
                nc.vector.tensor_reduce(
                    mAB.rearrange("p (h q) -> p h q", h=2),
                    pAB.rearrange("p (h b q) -> p h q b", h=2, b=BPI),
                    axis=mybir.AxisListType.X, op=OP.min)
                pT = psum_pool.tile([1, 32], FP32, tag="pT", name="pT")
                nc.tensor.matmul(pT[:], ones[:], mAB[:])
                fin = pool.tile([1, 32], FP32, tag="fin", name="fin")
                nc.scalar.copy(fin[:], pT[:])
                nc.sync.dma_start(out[:], fin[:], single_packet=True)

    nc.compile()
    return nc


# partition row p = b*16 + q, q = n*4 + c
_P = np.arange(128)
_B_IDX = _P // (NI * CH)
_N_IDX = (_P % (NI * CH)) // CH


def shard_inputs(boxes, fragments):
    """Per-core input marshalling (layout only, no arithmetic)."""
    boxes = np.ascontiguousarray(boxes, dtype=np.float32).reshape(
        N_CORES, NI, BPI, 4)
    frag = np.ascontiguousarray(fragments, dtype=np.float32).reshape(
        N_CORES, NI, CH, CW, 2)
    in_maps = []
    for k in range(N_CORES):
        f2 = frag[k].transpose(3, 0, 1, 2).reshape(2, ROWS, CW)
        frag2 = np.ascontiguousarray(
            np.broadcast_to(f2[:, None], (2, BPI, ROWS, CW)).reshape(2, 128, CW))
        boxp = np.ascontiguousarray(boxes[k, _N_IDX, _B_IDX, :])
        in_maps.append({"frag2": frag2, "boxp": boxp})
    return in_maps


_NC = None


def _get_nc():
    global _NC
    if _NC is None:
        _patch_walrus_args()
        _NC = build_nc()
    return _NC


def run(boxes, fragments, trace=False, **spmd_kwargs):
    nc = _get_nc()
    in_maps = shard_inputs(boxes, fragments)
    res = run_bass_kernel_spmd(nc, in_maps, list(range(N_CORES)),
                               trace=trace, **spmd_kwargs)
    total = np.float32(sum(
        np.asarray(r["out"], dtype=np.float32).sum(dtype=np.float32)
        for r in res.results))
    loss = np.float32(total / np.float32(FP * N_IMG))
    return loss, res


def kernel(boxes, fragments, obj_to_img):
    loss, _ = run(boxes, fragments)
    return loss
